# revision 20
# baseline (speedup 1.0000x reference)
"""Trainium2 Bass kernel for nn_DS4DKernel_56504589746318.

Math (per batch b):
    deltaA = W @ du[b]              # (N=64, L=4096)
    S      = cumsum_L(deltaA)       # (64, 4096)  -- tensor_tensor_scan
    K[b]   = (C*Bvec) @ S + base    # (H=1024, L=4096), base = C @ (A @ Bvec)

Sharding: data-parallel over batch, one batch per NeuronCore (B=8 = 8 cores).
Small matrices (W^T, (C*Bvec)^T, base) are precomputed on host and replicated.

HBM-bound at ~16.3 MiB/core: du in and K out stream as bf16 (host casts du
f32->bf16, upcasts K bf16->f32).  PSUM accumulation and the cumsum carry are
f32; S and all matmul operands are bf16 (one PE dtype -- mixing f32r and
bf16 streams flushes the PE pipeline on every switch).  Total quantization
error ~1e-3 vs the 2e-2 gate.  All DMA descriptor runs are 4KB (2048-element
supertiles; 2KB runs cost the same ~150ns fixed descriptor time and halve
bandwidth).  Loads are HWDGE (sync queue, ~4us earlier first byte than
SWDGE).  Dummy matmuls during the load fill warm the PE HAM clock gate so
real matmuls run at 2.4GHz, not 1.2.  mm2 fuses both halves of an output
supertile per c-chunk (4 matmuls, 2 1024-wide PSUM->SBUF +base copies split
DVE/ACT, one 0.5MiB store), which paces stores at wire rate with no tail
burst.  The serial scan chain hides under mm1/mm2 PE work.
"""

import sys

for _p in ("/opt/trn_rl_repo", "/root/.axon_site/_ro/trn_rl_repo"):
    if _p not in sys.path:
        sys.path.insert(0, _p)

import ml_dtypes
import numpy as np

import concourse.bass as bass
import concourse.mybir as mybir
import concourse.tile as tile
from concourse import bacc
from concourse.bass_utils import run_bass_kernel_spmd

B, H, N, L = 8, 1024, 64, 4096
P = 128          # SBUF partitions
HC = H // P      # 8 h-chunks of 128
ST = 2048        # DMA supertile width (4KB contiguous bf16 descriptor runs)
NST = L // ST    # 2 supertiles
LT = 1024        # compute l-tile width (PSUM bank pair)
NLT = L // LT    # 4 l-tiles
MM_N = 512       # matmul moving free dim (one PSUM bank of f32)
NS = LT // MM_N  # N-subtiles per l-tile
NWARM = 18       # dummy matmuls to warm the PE HAM clock gate

F32 = mybir.dt.float32
BF16 = mybir.dt.bfloat16
ADD = mybir.AluOpType.add
BYPASS = mybir.AluOpType.bypass

BF16_NP = ml_dtypes.bfloat16


def build_nc():
    nc = bacc.Bacc()
    du_d = nc.declare_dram_parameter("du", [H, L], BF16, isOutput=False)
    wt_d = nc.declare_dram_parameter("wt", [H, N], BF16, isOutput=False)
    ccbt_d = nc.declare_dram_parameter("ccbt", [N, H], BF16, isOutput=False)
    base_d = nc.declare_dram_parameter("base", [P, HC], F32, isOutput=False)
    out_d = nc.declare_dram_parameter("out", [H, L], BF16, isOutput=True)

    with tile.TileContext(nc) as tc:
        with (
            tc.tile_pool(name="const", bufs=1) as cpool,
            tc.tile_pool(name="du", bufs=2) as dupool,
            tc.tile_pool(name="s", bufs=4) as spool,
            tc.tile_pool(name="outp", bufs=2) as opool,
            tc.tile_pool(name="psA", bufs=2, space="PSUM") as psA,
            tc.tile_pool(name="psB", bufs=4, space="PSUM") as psB,
        ):
            # --- constants first: tiny, must not queue ahead of du ---
            wt_sb = cpool.tile([P, HC, N], BF16)     # [p, c, n] = W^T[c*128+p, n]
            nc.sync.dma_start(
                wt_sb[:], wt_d[:, :].rearrange("(c p) n -> p c n", p=P)
            )
            base_sb = cpool.tile([P, HC], F32)       # [p, c] = base[c*128+p]
            nc.sync.dma_start(base_sb[:], base_d[:, :])
            ccbt_sb = cpool.tile([N, H], BF16)       # [n, h] = (C*Bvec)^T
            nc.sync.dma_start(ccbt_sb[:], ccbt_d[:, :])
            zeros_sb = cpool.tile([N, LT], F32)      # data1 for the scan
            nc.vector.memset(zeros_sb[:], 0.0)
            scratch = cpool.tile([P, 384], BF16)     # warmup matmul operands
            nc.vector.memset(scratch[:], 0.0)

            # --- input loads (HWDGE): st0 in c-pairs for early PE start ---
            du_t = [None] * NST
            du_t[0] = dupool.tile([P, HC, ST], BF16, tag="du_t", name="du_t")
            for g in range(4):
                nc.gpsimd.dma_start(
                    du_t[0][:, 2 * g : 2 * g + 2, :],
                    du_d[2 * g * P : (2 * g + 2) * P, 0:ST].rearrange(
                        "(c p) j -> p c j", p=P
                    ),
                )
            du_t[1] = dupool.tile([P, HC, ST], BF16, tag="du_t", name="du_t")
            for g in range(4):
                nc.gpsimd.dma_start(
                    du_t[1][:, 2 * g : 2 * g + 2, :],
                    du_d[2 * g * P : (2 * g + 2) * P, ST : 2 * ST].rearrange(
                        "(c p) j -> p c j", p=P
                    ),
                )

            # --- PE warmup: garbage matmuls feed the HAM activity monitor
            # while du streams in, so real matmuls run at full clock ---
            warm_po = psB.tile([P, MM_N], F32, tag="po", name="po")
            for _ in range(NWARM):
                nc.tensor.matmul(
                    warm_po[:, 0:256], scratch[:, 0:128], scratch[:, 128:384],
                    start=True, stop=True,
                )

            dA_t = [None] * NLT
            S_t = [None] * NLT
            out_sb = [None] * NST

            def mm1_group(st, c):
                # one c-chunk of deltaA for both halves of a supertile:
                # one LDWEIGHTS serves 4 matmuls; c<4 depends only on the
                # first half of the supertile load
                for t in (0, 1):
                    for s in range(NS):
                        lo = t * LT + s * MM_N
                        nc.tensor.matmul(
                            dA_t[2 * st + t][:, s * MM_N : (s + 1) * MM_N],
                            wt_sb[:, c, :],
                            du_t[st][:, c, lo : lo + MM_N],
                            start=(c == 0),
                            stop=(c == HC - 1),
                        )

            def scan(lt):
                # high priority: the serial carry chain gates downstream mm2
                # copies, so it must preempt queued copy work on the DVE the
                # moment dA lands
                S_t[lt] = spool.tile([N, LT], BF16, tag="S_t", name="S_t")
                initial = 0.0 if lt == 0 else S_t[lt - 1][:, LT - 1 : LT]
                with tc.high_priority():
                    nc.vector.tensor_tensor_scan(
                        S_t[lt][:], dA_t[lt][:], zeros_sb[:], initial,
                        op0=ADD, op1=BYPASS,
                    )

            def mm2_group(st, c, engs):
                # one c-chunk of an output supertile: 4 matmuls (one
                # LDWEIGHTS), 4 512-wide PSUM->SBUF(bf16) +base copies from
                # a 4-deep PSUM ring (PE never blocks on a copy), then one
                # 0.5MiB store with 4KB runs
                for q in range(4):
                    po = psB.tile([P, MM_N], F32, tag="po", name="po")
                    lt, s = 2 * st + q // 2, q % 2
                    nc.tensor.matmul(
                        po[:],
                        ccbt_sb[:, c * P : (c + 1) * P],
                        S_t[lt][:, s * MM_N : (s + 1) * MM_N],
                        start=True,
                        stop=True,
                    )
                    dst = out_sb[st][:, c, q * MM_N : (q + 1) * MM_N]
                    if engs[(c * 4 + q) % len(engs)] == "a":
                        nc.scalar.add(dst, po[:], base_sb[:, c : c + 1])
                    else:
                        nc.vector.tensor_scalar_add(
                            dst, po[:], base_sb[:, c : c + 1]
                        )
                nc.sync.dma_start(
                    out_d[c * P : (c + 1) * P, st * ST : (st + 1) * ST],
                    out_sb[st][:, c, :],
                )

            # Phase A: mm1 of supertile 0 (c-groups fire as load c-pairs
            # land), then scans 0/1 on the DVE.
            dA_t[0] = psA.tile([N, LT], F32, tag="dA_t", name="dA_t")
            dA_t[1] = psA.tile([N, LT], F32, tag="dA_t", name="dA_t")
            for c in range(HC):
                mm1_group(0, c)
            scan(0)
            scan(1)

            # Phase B: mm1 of supertile 1 runs uninterrupted (purely load-
            # paced, so dA2/dA3 complete right after the last load lands),
            # scans 2/3 fire immediately on the DVE, and only then does the
            # mm2 copy/store stream begin.  mm2(st0) copies go all-ACT so
            # the DVE is free for the scans; mm2(st1) copies are DVE-heavy
            # to rebalance total copy work between the two engines.
            dA_t[2] = psA.tile([N, LT], F32, tag="dA_t", name="dA_t")
            dA_t[3] = psA.tile([N, LT], F32, tag="dA_t", name="dA_t")
            out_sb[0] = opool.tile([P, HC, ST], BF16, tag="o", name="o")
            for i in range(HC // 2):
                mm1_group(1, 2 * i)
                mm1_group(1, 2 * i + 1)
                mm2_group(0, i, engs="avavavav")
            for c in range(HC // 2, HC):
                mm2_group(0, c, engs="avavavav")
            # scans 2/3 are gated by dA2/dA3 (last st1 load) anyway
            scan(2)
            scan(3)

            # Phase C: mm2+stores of supertile 1.
            out_sb[1] = opool.tile([P, HC, ST], BF16, tag="o", name="o")
            for c in range(HC):
                mm2_group(1, c, engs="avavavav")

    nc.compile()
    return nc


_NC_CACHE = None


def _get_nc():
    global _NC_CACHE
    if _NC_CACHE is None:
        _NC_CACHE = build_nc()
    return _NC_CACHE


def _prep_in_maps(du, C, Bvec, A, W):
    du = np.asarray(du, dtype=np.float32)
    C = np.asarray(C, dtype=np.float32)
    Bvec = np.asarray(Bvec, dtype=np.float32)
    A = np.asarray(A, dtype=np.float32)
    W = np.asarray(W, dtype=np.float32)

    du_bf = np.ascontiguousarray(du.astype(BF16_NP))    # (B, H, L) bf16
    wt = np.ascontiguousarray(W.T.astype(BF16_NP))      # (H, N) bf16
    ccbt = np.ascontiguousarray(
        (C * Bvec[None, :]).T.astype(BF16_NP)           # (N, H) bf16
    )
    base = C @ (A @ Bvec)                               # (H,)
    base_t = np.ascontiguousarray(base.reshape(HC, P).T)  # (P, HC)

    return [
        {"du": du_bf[b], "wt": wt, "ccbt": ccbt, "base": base_t}
        for b in range(B)
    ]


def run(du, C, Bvec, A, W, trace=False):
    nc = _get_nc()
    in_maps = _prep_in_maps(du, C, Bvec, A, W)
    res = run_bass_kernel_spmd(nc, in_maps, core_ids=list(range(B)), trace=trace)
    out = np.stack(
        [res.results[b]["out"].astype(np.float32) for b in range(B)], axis=0
    )
    return out, res


def kernel(du, C, Bvec, A, W):
    out, _ = run(du, C, Bvec, A, W, trace=False)
    return out
